# revision 1
# baseline (speedup 1.0000x reference)
"""Trainium2 Bass kernel for nn_LocalEnergyCore (sampling / local energy MLP).

Contract: kernel(**inputs) takes FULL unsharded inputs, returns FULL output
(scalar f32). Internally shards z along batch across 8 NeuronCores.

Per-core device program (indices are baked into the program at build time):
  - z shard is fed as [K=8, H+2=66, W+2=66, B_loc=512] bf16 with toroidal halo,
    batch innermost -> every (k, row) line is 1KB contiguous.
  - For each of the 50 sites: one DMA gathers the 3x3xK neighborhood as a
    [72, 512] SBUF tile (partition = (di, dj, k), free = batch).
  - L1: per-site matmul with one of 8 "variant" W1 matrices [72, 64] (the
    dropped center-self row folded in as an exact zero row). Two sites share
    one [128, 512] PSUM tile (partition offsets 0 / 64).
  - ACT: relu(h + b1) -> bf16 SBUF.
  - L2: 25 accumulated matmuls with block-diagonal W2 columns collect all 50
    logits into ONE [50, 512] PSUM tile.
  - One DVE scalar_tensor_tensor: ((logit > -b2) != target) summed over batch
    -> counts [50, 1]; target rows were DMA-gathered (bf16->f32 cast on SWDGE).
  - ones-matmul reduces counts across partitions; ACT scales by 1/(B*S).
Host sums the 8 per-core partial means.
"""

import sys

for _p in ("/opt/trn_rl_repo",):
    if _p not in sys.path:
        sys.path.insert(0, _p)

import numpy as np
import ml_dtypes

B, K, H, W = 4096, 8, 64, 64
S, HID, CTX = 50, 64, 71
N_CORES = 8
B_LOC = B // N_CORES

BF16 = ml_dtypes.bfloat16

LAST_RESULTS = None  # test harness introspection


def _host_prep(z, W1, b1, W2, b2, b_idx, i_idx, j_idx):
    """Shard + lay out inputs; returns (in_maps, site list, -b2)."""
    b_idx = np.asarray(b_idx).astype(np.int64)
    i_idx = np.asarray(i_idx).astype(np.int64)
    j_idx = np.asarray(j_idx).astype(np.int64)

    # sites sorted by variant (stationary-weight locality on PE)
    order = np.argsort(b_idx, kind="stable")
    sites = [(int(b_idx[s]), int(i_idx[s]), int(j_idx[s])) for s in order]

    # variant W1 matrices: [72, HID] with row t = W1[t - (t > drop)], row drop = 0.
    # Rows are then permuted to the gather order c = di*24 + k*3 + dj
    # (original order is position-major: c0 = (3*di+dj)*8 + k).
    W1V = np.zeros((K, 72, HID), dtype=np.float32)
    t = np.arange(72)
    for v in range(K):
        drop = 4 * K + v
        src = t - (t > drop)
        W1V[v] = W1[np.minimum(src, CTX - 1)]
        W1V[v, drop] = 0.0
    perm = np.empty(72, dtype=np.int64)
    for di in range(3):
        for k in range(K):
            for dj in range(3):
                perm[di * 24 + k * 3 + dj] = (3 * di + dj) * K + k
    W1V = W1V[:, perm, :]
    # device layout: [72, K*HID], columns v*64:(v+1)*64 = variant v
    w1v_np = np.ascontiguousarray(W1V.transpose(1, 0, 2).reshape(72, K * HID)).astype(BF16)

    # block-diagonal W2 for the accumulated L2 collect: [128, 25*S]
    w2big = np.zeros((128, S // 2, S), dtype=np.float32)
    for p in range(S // 2):
        w2big[0:HID, p, 2 * p] = W2
        w2big[HID:128, p, 2 * p + 1] = W2
    w2big_np = np.ascontiguousarray(w2big.reshape(128, (S // 2) * S)).astype(BF16)

    bias1_np = np.concatenate([b1, b1]).astype(np.float32).reshape(128, 1)

    # indirect-gather index tables (row units of the flattened [K*66*66, B_LOC] z)
    # ctx row (c, s): (di, dj, k) = inv of c = di*24 + k*3 + dj
    gidx = np.zeros((72, S), dtype=np.int32)
    for c in range(72):
        di, rem = divmod(c, 24)
        k, dj = divmod(rem, 3)
        for s, (v, i0, j0) in enumerate(sites):
            gidx[c, s] = k * 66 * 66 + (i0 + di) * 66 + (j0 + dj)
    tidx = np.array([[v * 66 * 66 + (1 + i0) * 66 + (1 + j0)]
                     for (v, i0, j0) in sites], dtype=np.int32)

    # z shards: [K, H+2, W+2, B_loc] bf16 with wrap halo, batch innermost
    in_maps = []
    zb = z.astype(BF16)
    for c in range(N_CORES):
        zt = zb[c * B_LOC:(c + 1) * B_LOC].transpose(1, 2, 3, 0)  # [K,H,W,Bl]
        zp = np.empty((K, H + 2, W + 2, B_LOC), dtype=BF16)
        zp[:, 1:H + 1, 1:W + 1, :] = zt
        zp[:, 0, 1:W + 1, :] = zt[:, H - 1]
        zp[:, H + 1, 1:W + 1, :] = zt[:, 0]
        zp[:, :, 0, :] = zp[:, :, W, :]
        zp[:, :, W + 1, :] = zp[:, :, 1, :]
        in_maps.append({
            "z": np.ascontiguousarray(zp),
            "w1v": w1v_np,
            "w2big": w2big_np,
            "bias1": bias1_np,
            "gidx": gidx,
            "tidx": tidx,
        })
    return in_maps, sites, -float(np.asarray(b2))


def _build_program(sites, neg_b2):
    """Emit the per-core Bass program (identical across cores)."""
    import concourse.bacc as bacc
    import concourse.mybir as mybir
    import concourse.tile as tile

    fp32 = mybir.dt.float32
    bf16 = mybir.dt.bfloat16

    nc = bacc.Bacc("TRN2", target_bir_lowering=False, debug=False,
                   num_devices=N_CORES)

    zin = nc.dram_tensor("z", [K, H + 2, W + 2, B_LOC], bf16, kind="ExternalInput")
    w1v_d = nc.dram_tensor("w1v", [72, K * HID], bf16, kind="ExternalInput")
    w2b_d = nc.dram_tensor("w2big", [128, (S // 2) * S], bf16, kind="ExternalInput")
    b1_d = nc.dram_tensor("bias1", [128, 1], fp32, kind="ExternalInput")
    outp = nc.dram_tensor("out", [1, 1], fp32, kind="ExternalOutput")

    with tile.TileContext(nc) as tc:
        with (
            tc.tile_pool(name="const", bufs=1) as cpool,
            tc.tile_pool(name="ctx", bufs=50) as ctxpool,
            tc.tile_pool(name="hsb", bufs=4) as hpool,
            tc.tile_pool(name="ps", bufs=4, space="PSUM") as pspool,
            tc.tile_pool(name="psl", bufs=1, space="PSUM") as pslpool,
            tc.tile_pool(name="pst", bufs=1, space="PSUM") as pstpool,
        ):
            w1v_sb = cpool.tile([72, K * HID], bf16)
            nc.sync.dma_start(out=w1v_sb[:, :], in_=w1v_d[:, :])
            w2b_sb = cpool.tile([128, (S // 2) * S], bf16)
            nc.sync.dma_start(out=w2b_sb[:, :], in_=w2b_d[:, :])
            b1_sb = cpool.tile([128, 1], fp32)
            nc.sync.dma_start(out=b1_sb[:, :], in_=b1_d[:, :])
            ones_sb = cpool.tile([S, 1], fp32)
            nc.vector.memset(ones_sb[:, :], 1.0)

            # target rows: center of each site's patch (bf16, cast once on DVE)
            t_bf = cpool.tile([S, B_LOC], bf16)
            for r, (v, i0, j0) in enumerate(sites):
                nc.sync.dma_start(out=t_bf[r:r + 1, :],
                                  in_=zin[v, 1 + i0, 1 + j0, :])
            t32 = cpool.tile([S, B_LOC], fp32)
            nc.vector.tensor_copy(out=t32[:, :], in_=t_bf[:, :])

            logit_ps = pslpool.tile([S, B_LOC], fp32)

            # gather + L1 + relu + accumulated L2, software-pipelined
            def emit_pair(p):
                h_ps = pspool.tile([128, B_LOC], fp32, tag="hps")
                ctx_t = []
                for q in (0, 1):
                    v, i0, j0 = sites[2 * p + q]
                    ct = ctxpool.tile([72, B_LOC], bf16, tag="ctx")
                    for di in range(3):
                        # [K, 3, B_LOC] (k, dj, b) -> partitions di*24 + k*3 + dj
                        eng = nc.scalar if di == 1 else nc.sync
                        eng.dma_start(out=ct[di * 24:(di + 1) * 24, :],
                                      in_=zin[:, i0 + di, j0:j0 + 3, :])
                    ctx_t.append((ct, v))
                for q in (0, 1):
                    ct, v = ctx_t[q]
                    nc.tensor.matmul(
                        h_ps[q * HID:(q + 1) * HID, :],
                        w1v_sb[:, v * HID:(v + 1) * HID],
                        ct[:, :],
                        start=True, stop=True)
                h_sb = hpool.tile([128, B_LOC], bf16, tag="hsb")
                nc.scalar.activation(
                    out=h_sb[:, :], in_=h_ps[:, :],
                    func=mybir.ActivationFunctionType.Relu,
                    bias=b1_sb[:, :], scale=1.0)
                return h_sb

            n_pairs = S // 2
            pend = None  # (h_sb, p) awaiting its L2
            for p in range(n_pairs):
                h_sb = emit_pair(p)
                if pend is not None:
                    hs, pp = pend
                    nc.tensor.matmul(
                        logit_ps[:, :],
                        w2b_sb[:, pp * S:(pp + 1) * S],
                        hs[:, :],
                        start=(pp == 0), stop=False)
                pend = (h_sb, p)
            hs, pp = pend
            nc.tensor.matmul(
                logit_ps[:, :],
                w2b_sb[:, pp * S:(pp + 1) * S],
                hs[:, :],
                start=(pp == 0), stop=True)

            # ((logit > -b2) != target), summed over batch -> counts [S, 1]
            junk = cpool.tile([S, B_LOC], fp32)
            counts = cpool.tile([S, 1], fp32)
            nc.vector.scalar_tensor_tensor(
                out=junk[:, :], in0=logit_ps[:, :], scalar=neg_b2,
                in1=t32[:, :],
                op0=mybir.AluOpType.is_gt, op1=mybir.AluOpType.not_equal,
                accum_out=counts[:, :])

            tot_ps = pstpool.tile([1, 1], fp32, tag="tot")
            nc.tensor.matmul(tot_ps[:, :], ones_sb[:, :], counts[:, :],
                             start=True, stop=True)
            res_sb = cpool.tile([1, 1], fp32)
            nc.scalar.activation(out=res_sb[:, :], in_=tot_ps[:, :],
                                 func=mybir.ActivationFunctionType.Copy,
                                 scale=1.0 / float(B * S))
            nc.sync.dma_start(out=outp[:, :], in_=res_sb[:, :])

    nc.compile()
    return nc


def kernel(**inputs):
    global LAST_RESULTS
    from concourse.bass_utils import run_bass_kernel_spmd

    z = np.asarray(inputs["z"], dtype=np.float32)
    in_maps, sites, neg_b2 = _host_prep(
        z, np.asarray(inputs["W1"], dtype=np.float32),
        np.asarray(inputs["b1"], dtype=np.float32),
        np.asarray(inputs["W2"], dtype=np.float32),
        inputs["b2"], inputs["b_idx"], inputs["i_idx"], inputs["j_idx"])

    nc = _build_program(sites, neg_b2)

    res = run_bass_kernel_spmd(nc, in_maps, list(range(N_CORES)))
    LAST_RESULTS = res
    total = np.float32(0.0)
    for r in res.results:
        total += np.float32(r["out"][0, 0])
    return np.float32(total)



# revision 2
# speedup vs baseline: 1.1183x; 1.1183x over previous
"""Trainium2 Bass kernel for nn_LocalEnergyCore — v2.

Differences vs baseline:
  - z shards stored [H+2, W+2, K, B_loc] in fp8e4m3 (bits 0/1 are exact in
    fp8): gather DMA bytes halve, descriptors stay at the 512B line-rate
    minimum.
  - ONE gather DMA per site ([3,3,8,512] block -> [72, 512] ctx tile,
    partition order c = di*24 + dj*8 + k) instead of 3.
  - The 50 target rows come via a single SWDGE indirect_dma_start driven by
    an int32 row-index table (one instruction, 50 descriptors).
  - L1 pair matmuls carry explicit tile_position (0,0)/(0,64) so the two
    sites of a pair run concurrently on the PE's column halves.
  - relu+bias alternates between DVE (tensor_scalar add+max, per-partition
    bias AP) and ACT (activation Relu with bias) so neither engine is the
    bottleneck.
  - L2 collect is split into TWO accumulation chains in different PSUM banks
    (even pairs -> bank A rows 0:26 at col tile 0, odd pairs -> bank B rows
    64:88 at col tile 64); chains run concurrently on the PE column halves.
    Each accumulator holds a disjoint contiguous set of sites' logits, so the
    compare+count runs once per accumulator (a PSUM in0 + SBUF target in1)
    with no cross-bank add.
"""

import sys

for _p in ("/opt/trn_rl_repo",):
    if _p not in sys.path:
        sys.path.insert(0, _p)

import numpy as np
import ml_dtypes

B, K, H, W = 4096, 8, 64, 64
S, HID, CTX = 50, 64, 71
N_CORES = 8
B_LOC = B // N_CORES
NPAIR = S // 2

BF16 = ml_dtypes.bfloat16
FP8 = ml_dtypes.float8_e4m3fn

LAST_RESULTS = None  # test harness introspection


def _host_prep(z, W1, b1, W2, b2, b_idx, i_idx, j_idx):
    """Shard + lay out inputs; returns (in_maps, site list, -b2)."""
    b_idx = np.asarray(b_idx).astype(np.int64)
    i_idx = np.asarray(i_idx).astype(np.int64)
    j_idx = np.asarray(j_idx).astype(np.int64)
    sites = [(int(b_idx[s]), int(i_idx[s]), int(j_idx[s])) for s in range(S)]

    # variant W1 matrices: [72, HID] with row t = W1[t - (t > drop)], row
    # drop = 0. Rows permuted to the gather order c = di*24 + dj*8 + k
    # (original order is position-major: c0 = (3*di+dj)*8 + k).
    W1V = np.zeros((K, 72, HID), dtype=np.float32)
    t = np.arange(72)
    for v in range(K):
        drop = 4 * K + v
        src = t - (t > drop)
        W1V[v] = W1[np.minimum(src, CTX - 1)]
        W1V[v, drop] = 0.0
    perm = np.empty(72, dtype=np.int64)
    for di in range(3):
        for dj in range(3):
            for k in range(K):
                perm[di * 24 + dj * 8 + k] = (3 * di + dj) * K + k
    W1V = W1V[:, perm, :]
    w1v_np = np.ascontiguousarray(
        W1V.transpose(1, 0, 2).reshape(72, K * HID)).astype(FP8)

    # block-diagonal W2 for the two accumulated L2 chains. Even pairs
    # (p=0,2,..,24) collect into chain A compare-rows 0:26; odd pairs into
    # chain B compare-rows 32:56 (engine partition bases must be 32-aligned).
    # cmp_row[s] = site s's row in the packed [56] compare layout.
    NA = 2 * ((NPAIR + 1) // 2)       # 26 chain-A rows
    NB = S - NA                       # 24 chain-B rows
    NC = 32 + NB                      # compare layout height (56)
    cmp_row = np.zeros(S, dtype=np.int64)
    w2a = np.zeros((128, (NPAIR + 1) // 2, NA), dtype=np.float32)
    w2b = np.zeros((128, NPAIR // 2, NB), dtype=np.float32)
    for p in range(NPAIR):
        for q in (0, 1):
            s = 2 * p + q
            if p % 2 == 0:
                loc = 2 * (p // 2) + q
                w2a[q * HID:(q + 1) * HID, p // 2, loc] = W2
                cmp_row[s] = loc
            else:
                loc = 2 * ((p - 1) // 2) + q
                w2b[q * HID:(q + 1) * HID, (p - 1) // 2, loc] = W2
                cmp_row[s] = 32 + loc
    w2a_np = np.ascontiguousarray(
        w2a.reshape(128, ((NPAIR + 1) // 2) * NA)).astype(BF16)
    w2b_np = np.ascontiguousarray(
        w2b.reshape(128, (NPAIR // 2) * NB)).astype(BF16)

    # pack w2a|w2b into one DMA payload
    w2pack = np.concatenate([w2a_np, w2b_np], axis=1)

    # target row index per compare-row into z flattened [(i, j, k), b];
    # onesvec masks the unused rows 26:32 out of the final reduction.
    # Pack bias1 / onesvec / tidx(bit-cast) into one [128, 3] f32 payload.
    tidx = np.zeros((NC, 1), dtype=np.int32)
    onesvec = np.zeros((NC, 1), dtype=np.float32)
    for s, (v, i0, j0) in enumerate(sites):
        tidx[cmp_row[s], 0] = ((1 + i0) * 66 + (1 + j0)) * K + v
        onesvec[cmp_row[s], 0] = 1.0
    smallpack = np.zeros((128, 3), dtype=np.float32)
    smallpack[:, 0] = np.concatenate([b1, b1]).astype(np.float32)
    smallpack[0:NC, 1] = onesvec[:, 0]
    smallpack[0:NC, 2] = tidx[:, 0].view(np.float32)

    # z shards: [H+2, W+2, K, B_loc] fp8 with wrap halo, batch innermost
    in_maps = []
    z8 = z.astype(FP8)
    for c in range(N_CORES):
        zt = z8[c * B_LOC:(c + 1) * B_LOC].transpose(2, 3, 1, 0)  # [H,W,K,Bl]
        zp = np.empty((H + 2, W + 2, K, B_LOC), dtype=FP8)
        zp[1:H + 1, 1:W + 1] = zt
        zp[0, 1:W + 1] = zt[H - 1]
        zp[H + 1, 1:W + 1] = zt[0]
        zp[:, 0] = zp[:, W]
        zp[:, W + 1] = zp[:, 1]
        in_maps.append({
            "z": np.ascontiguousarray(zp),
            "w1v": w1v_np,
            "w2pack": w2pack,
            "smallpack": smallpack,
        })
    return in_maps, sites, -float(np.asarray(b2))


def _build_program(sites, neg_b2):
    """Emit the per-core Bass program (identical across cores)."""
    import concourse.bacc as bacc
    import concourse.bass as bass
    import concourse.mybir as mybir
    import concourse.tile as tile

    fp32 = mybir.dt.float32
    bf16 = mybir.dt.bfloat16
    fp8 = mybir.dt.float8e4
    i32 = mybir.dt.int32

    nc = bacc.Bacc("TRN2", target_bir_lowering=False, debug=False,
                   num_devices=N_CORES)

    NA = 2 * ((NPAIR + 1) // 2)       # 26 chain-A rows
    NB = S - NA                       # 24 chain-B rows
    NC = 32 + NB                      # compare layout height (56)

    NW2A = ((NPAIR + 1) // 2) * NA
    NW2B = (NPAIR // 2) * NB

    zin = nc.dram_tensor("z", [H + 2, W + 2, K, B_LOC], fp8,
                         kind="ExternalInput")
    w1v_d = nc.dram_tensor("w1v", [72, K * HID], fp8, kind="ExternalInput")
    w2p_d = nc.dram_tensor("w2pack", [128, NW2A + NW2B], bf16,
                           kind="ExternalInput")
    sp_d = nc.dram_tensor("smallpack", [128, 3], fp32, kind="ExternalInput")
    outp = nc.dram_tensor("out", [1, 1], fp32, kind="ExternalOutput")

    with tile.TileContext(nc) as tc:
        with (
            tc.tile_pool(name="const", bufs=1) as cpool,
            tc.tile_pool(name="ctx", bufs=24) as ctxpool,
            tc.tile_pool(name="hsb", bufs=8) as hpool,
            tc.tile_pool(name="ps", bufs=4, space="PSUM") as pspool,
            tc.tile_pool(name="psl", bufs=1, space="PSUM") as pslpool,
            tc.tile_pool(name="pst", bufs=1, space="PSUM") as pstpool,
        ):
            def emit_gather(s):
                v, i0, j0 = sites[s]
                ct = ctxpool.tile([72, B_LOC], fp8, tag="ctx")
                # ~20/20/10 split across the two HWDGE rings + SWDGE ring
                eng = (nc.sync, nc.scalar, nc.sync, nc.scalar,
                       nc.gpsimd)[s % 5]
                eng.dma_start(out=ct[:, :], in_=zin[i0:i0 + 3, j0:j0 + 3, :, :])
                return ct

            # critical path first: L1 weights + the first pairs' gathers
            w1v_sb = cpool.tile([72, K * HID], fp8)
            nc.sync.dma_start(out=w1v_sb[:, :], in_=w1v_d[:, :])
            ctx_q = {}
            for s in range(6):
                ctx_q[s] = emit_gather(s)

            # remaining constants (needed a few microseconds in)
            sp_sb = cpool.tile([128, 3], fp32)
            nc.scalar.dma_start(out=sp_sb[:, :], in_=sp_d[:, :])
            b1_sb = sp_sb[:, 0:1]
            ones_ap = sp_sb[0:NC, 1:2]
            tidx_ap = sp_sb[0:NC, 2:3].bitcast(i32)
            w2p_sb = cpool.tile([128, NW2A + NW2B], bf16)
            nc.scalar.dma_start(out=w2p_sb[:, :], in_=w2p_d[:, :])
            w2a_sb = w2p_sb[:, 0:NW2A]
            w2b_sb = w2p_sb[:, NW2A:NW2A + NW2B]

            # all target rows in one indirect gather (partitions 64+ to
            # touch the otherwise-idle high SBUF ports)
            tstage = cpool.tile([128, B_LOC], fp8)
            zflat = zin[:, :, :, :].rearrange("i j k b -> (i j k) b")
            nc.gpsimd.indirect_dma_start(
                out=tstage[64:64 + NC, :],
                out_offset=None,
                in_=zflat,
                in_offset=bass.IndirectOffsetOnAxis(ap=tidx_ap, axis=0),
            )
            t32 = cpool.tile([NC, B_LOC], fp32)
            nc.vector.tensor_copy(out=t32[:, :], in_=tstage[64:64 + NC, :])

            # two L2 accumulators in separate PSUM banks (interleaved
            # accumulation groups cannot share a bank: start=True clears the
            # whole bank's has_written bits)
            logA = pslpool.tile([128, B_LOC], fp32, tag="logA")
            logB = pslpool.tile([128, B_LOC], fp32, tag="logB")

            def emit_l2(pp, hs):
                if pp % 2 == 0:
                    e = pp // 2
                    outsl = logA[0:NA, :]
                    wsl = w2a_sb[:, e * NA:(e + 1) * NA]
                    col = 0
                else:
                    e = (pp - 1) // 2
                    outsl = logB[64:64 + NB, :]
                    wsl = w2b_sb[:, e * NB:(e + 1) * NB]
                    col = 64
                nc.tensor.matmul(
                    outsl, wsl, hs[:, :],
                    start=(pp in (0, 1)),
                    stop=(pp in (NPAIR - 1, NPAIR - 2)),
                    tile_position=(0, col),
                    skip_group_check=True)

            PF = 10  # pairs of gather prefetch
            for s in range(6, 2 * PF):
                ctx_q[s] = emit_gather(s)

            hq = {}
            for p in range(NPAIR):
                sA, sB = 2 * p, 2 * p + 1
                ctA = ctx_q.pop(sA)
                ctB = ctx_q.pop(sB)
                vA = sites[sA][0]
                vB = sites[sB][0]
                h_ps = pspool.tile([128, B_LOC], fp32, tag="hps")
                nc.tensor.matmul(
                    h_ps[0:HID, :], w1v_sb[:, vA * HID:(vA + 1) * HID],
                    ctA[:, :], start=True, stop=True, tile_position=(0, 0))
                nc.tensor.matmul(
                    h_ps[HID:128, :], w1v_sb[:, vB * HID:(vB + 1) * HID],
                    ctB[:, :], start=True, stop=True, tile_position=(0, 64))
                for s in (2 * (p + PF), 2 * (p + PF) + 1):
                    if s < S:
                        ctx_q[s] = emit_gather(s)
                h_sb = hpool.tile([128, B_LOC], bf16, tag="hsb")
                if p % 5 == 4:
                    nc.scalar.activation(
                        out=h_sb[:, :], in_=h_ps[:, :],
                        func=mybir.ActivationFunctionType.Relu,
                        bias=b1_sb[:, 0:1], scale=1.0)
                else:
                    nc.vector.tensor_scalar(
                        out=h_sb[:, :], in0=h_ps[:, :],
                        scalar1=b1_sb[:, 0:1], scalar2=0.0,
                        op0=mybir.AluOpType.add, op1=mybir.AluOpType.max)
                hq[p] = h_sb
                if p % 2 == 1:
                    emit_l2(p - 1, hq.pop(p - 1))
                    emit_l2(p, hq.pop(p))
            for pp in sorted(hq):
                emit_l2(pp, hq.pop(pp))

            # ((logit > -b2) != target) summed over batch, per chain
            junk = cpool.tile([NC, B_LOC], fp32)
            counts = cpool.tile([NC, 1], fp32)
            nc.vector.memset(counts[:, :], 0.0)
            nc.vector.scalar_tensor_tensor(
                out=junk[0:NA, :], in0=logA[0:NA, :], scalar=neg_b2,
                in1=t32[0:NA, :],
                op0=mybir.AluOpType.is_gt, op1=mybir.AluOpType.not_equal,
                accum_out=counts[0:NA, :])
            nc.vector.scalar_tensor_tensor(
                out=junk[32:NC, :], in0=logB[64:64 + NB, :], scalar=neg_b2,
                in1=t32[32:NC, :],
                op0=mybir.AluOpType.is_gt, op1=mybir.AluOpType.not_equal,
                accum_out=counts[32:NC, :])

            tot_ps = pstpool.tile([1, 1], fp32, tag="tot")
            nc.tensor.matmul(tot_ps[:, :], ones_ap, counts[:, :],
                             start=True, stop=True)
            res_sb = cpool.tile([1, 1], fp32)
            nc.vector.tensor_scalar_mul(res_sb[:, :], tot_ps[:, :],
                                        1.0 / float(B * S))
            nc.sync.dma_start(out=outp[:, :], in_=res_sb[:, :])

    nc.compile()
    return nc


def kernel(**inputs):
    global LAST_RESULTS
    from concourse.bass_utils import run_bass_kernel_spmd

    z = np.asarray(inputs["z"], dtype=np.float32)
    in_maps, sites, neg_b2 = _host_prep(
        z, np.asarray(inputs["W1"], dtype=np.float32),
        np.asarray(inputs["b1"], dtype=np.float32),
        np.asarray(inputs["W2"], dtype=np.float32),
        inputs["b2"], inputs["b_idx"], inputs["i_idx"], inputs["j_idx"])

    nc = _build_program(sites, neg_b2)

    res = run_bass_kernel_spmd(nc, in_maps, list(range(N_CORES)))
    LAST_RESULTS = res
    total = np.float32(0.0)
    for r in res.results:
        total += np.float32(r["out"][0, 0])
    return np.float32(total)


# revision 3
# speedup vs baseline: 1.1912x; 1.0652x over previous
"""Trainium2 Bass kernel for nn_LocalEnergyCore (sampling / local energy MLP).

Differences vs baseline:
  - z shards stored [H+2, W+2, K, B_loc] in fp8e4m3 (bits 0/1 are exact in
    fp8): gather DMA bytes halve, descriptors stay at the 512B line-rate
    minimum.
  - ONE gather DMA per site ([3,3,8,512] block -> [72, 512] ctx tile,
    partition order c = di*24 + dj*8 + k) instead of 3.
  - The 50 target rows come via a single SWDGE indirect_dma_start driven by
    an int32 row-index table (one instruction, 50 descriptors).
  - L1 pair matmuls carry explicit tile_position (0,0)/(0,64) so the two
    sites of a pair run concurrently on the PE's column halves.
  - relu+bias alternates between DVE (tensor_scalar add+max, per-partition
    bias AP) and ACT (activation Relu with bias) so neither engine is the
    bottleneck.
  - L2 collect is split into TWO accumulation chains in different PSUM banks
    (even pairs -> bank A rows 0:26 at col tile 0, odd pairs -> bank B rows
    64:88 at col tile 64); chains run concurrently on the PE column halves.
    Each accumulator holds a disjoint contiguous set of sites' logits, so the
    compare+count runs once per accumulator (a PSUM in0 + SBUF target in1)
    with no cross-bank add.
"""

import sys

for _p in ("/opt/trn_rl_repo",):
    if _p not in sys.path:
        sys.path.insert(0, _p)

import numpy as np
import ml_dtypes

B, K, H, W = 4096, 8, 64, 64
S, HID, CTX = 50, 64, 71
N_CORES = 8
B_LOC = B // N_CORES
NPAIR = S // 2

BF16 = ml_dtypes.bfloat16
FP8 = ml_dtypes.float8_e4m3fn

LAST_RESULTS = None  # test harness introspection


def _host_prep(z, W1, b1, W2, b2, b_idx, i_idx, j_idx):
    """Shard + lay out inputs; returns (in_maps, site list, -b2)."""
    b_idx = np.asarray(b_idx).astype(np.int64)
    i_idx = np.asarray(i_idx).astype(np.int64)
    j_idx = np.asarray(j_idx).astype(np.int64)
    sites = [(int(b_idx[s]), int(i_idx[s]), int(j_idx[s])) for s in range(S)]

    # variant W1 matrices: [72, HID] with row t = W1[t - (t > drop)], row
    # drop = 0. Rows permuted to the gather order c = di*24 + dj*8 + k
    # (original order is position-major: c0 = (3*di+dj)*8 + k).
    W1V = np.zeros((K, 72, HID), dtype=np.float32)
    t = np.arange(72)
    for v in range(K):
        drop = 4 * K + v
        src = t - (t > drop)
        W1V[v] = W1[np.minimum(src, CTX - 1)]
        W1V[v, drop] = 0.0
    perm = np.empty(72, dtype=np.int64)
    for di in range(3):
        for dj in range(3):
            for k in range(K):
                perm[di * 24 + dj * 8 + k] = (3 * di + dj) * K + k
    W1V = W1V[:, perm, :]
    w1v_np = np.ascontiguousarray(
        W1V.transpose(1, 0, 2).reshape(72, K * HID)).astype(FP8)

    # block-diagonal W2 for the two accumulated L2 chains. Even pairs
    # (p=0,2,..,24) collect into chain A compare-rows 0:26; odd pairs into
    # chain B compare-rows 32:56 (engine partition bases must be 32-aligned).
    # cmp_row[s] = site s's row in the packed [56] compare layout.
    NA = 2 * ((NPAIR + 1) // 2)       # 26 chain-A rows
    NB = S - NA                       # 24 chain-B rows
    NC = 32 + NB                      # compare layout height (56)
    cmp_row = np.zeros(S, dtype=np.int64)
    w2a = np.zeros((128, (NPAIR + 1) // 2, NA), dtype=np.float32)
    w2b = np.zeros((128, NPAIR // 2, NB), dtype=np.float32)
    for p in range(NPAIR):
        for q in (0, 1):
            s = 2 * p + q
            if p % 2 == 0:
                loc = 2 * (p // 2) + q
                w2a[q * HID:(q + 1) * HID, p // 2, loc] = W2
                cmp_row[s] = loc
            else:
                loc = 2 * ((p - 1) // 2) + q
                w2b[q * HID:(q + 1) * HID, (p - 1) // 2, loc] = W2
                cmp_row[s] = 32 + loc
    w2a_np = np.ascontiguousarray(
        w2a.reshape(128, ((NPAIR + 1) // 2) * NA)).astype(BF16)
    w2b_np = np.ascontiguousarray(
        w2b.reshape(128, (NPAIR // 2) * NB)).astype(BF16)

    # pack w2a|w2b into one DMA payload
    w2pack = np.concatenate([w2a_np, w2b_np], axis=1)

    # target row index per compare-row into z flattened [(i, j, k), b];
    # onesvec masks the unused rows 26:32 out of the final reduction.
    # Pack bias1 / onesvec / tidx(bit-cast) into one [128, 3] f32 payload.
    tidx = np.zeros((NC, 1), dtype=np.int32)
    onesvec = np.zeros((NC, 1), dtype=np.float32)
    for s, (v, i0, j0) in enumerate(sites):
        tidx[cmp_row[s], 0] = ((1 + i0) * 66 + (1 + j0)) * K + v
        onesvec[cmp_row[s], 0] = 1.0
    smallpack = np.zeros((128, 3), dtype=np.float32)
    smallpack[:, 0] = np.concatenate([b1, b1]).astype(np.float32)
    smallpack[0:NC, 1] = onesvec[:, 0]
    smallpack[0:NC, 2] = tidx[:, 0].view(np.float32)

    # z shards: [H+2, W+2, K, B_loc] fp8 with wrap halo, batch innermost
    in_maps = []
    z8 = z.astype(FP8)
    for c in range(N_CORES):
        zt = z8[c * B_LOC:(c + 1) * B_LOC].transpose(2, 3, 1, 0)  # [H,W,K,Bl]
        zp = np.empty((H + 2, W + 2, K, B_LOC), dtype=FP8)
        zp[1:H + 1, 1:W + 1] = zt
        zp[0, 1:W + 1] = zt[H - 1]
        zp[H + 1, 1:W + 1] = zt[0]
        zp[:, 0] = zp[:, W]
        zp[:, W + 1] = zp[:, 1]
        in_maps.append({
            "z": np.ascontiguousarray(zp),
            "w1v": w1v_np,
            "w2pack": w2pack,
            "smallpack": smallpack,
        })
    return in_maps, sites, -float(np.asarray(b2))


def _build_program(sites, neg_b2):
    """Emit the per-core Bass program (identical across cores)."""
    import concourse.bacc as bacc
    import concourse.bass as bass
    import concourse.mybir as mybir
    import concourse.tile as tile

    fp32 = mybir.dt.float32
    bf16 = mybir.dt.bfloat16
    fp8 = mybir.dt.float8e4
    i32 = mybir.dt.int32

    nc = bacc.Bacc("TRN2", target_bir_lowering=False, debug=False,
                   num_devices=N_CORES)

    NA = 2 * ((NPAIR + 1) // 2)       # 26 chain-A rows
    NB = S - NA                       # 24 chain-B rows
    NC = 32 + NB                      # compare layout height (56)

    NW2A = ((NPAIR + 1) // 2) * NA
    NW2B = (NPAIR // 2) * NB

    zin = nc.dram_tensor("z", [H + 2, W + 2, K, B_LOC], fp8,
                         kind="ExternalInput")
    w1v_d = nc.dram_tensor("w1v", [72, K * HID], fp8, kind="ExternalInput")
    w2p_d = nc.dram_tensor("w2pack", [128, NW2A + NW2B], bf16,
                           kind="ExternalInput")
    sp_d = nc.dram_tensor("smallpack", [128, 3], fp32, kind="ExternalInput")
    outp = nc.dram_tensor("out", [1, 1], fp32, kind="ExternalOutput")

    with tile.TileContext(nc) as tc:
        with (
            tc.tile_pool(name="const", bufs=1) as cpool,
            tc.tile_pool(name="ctx", bufs=24) as ctxpool,
            tc.tile_pool(name="hsb", bufs=8) as hpool,
            tc.tile_pool(name="ps", bufs=4, space="PSUM") as pspool,
            tc.tile_pool(name="psl", bufs=1, space="PSUM") as pslpool,
            tc.tile_pool(name="pst", bufs=1, space="PSUM") as pstpool,
        ):
            def emit_gather(s):
                v, i0, j0 = sites[s]
                ct = ctxpool.tile([72, B_LOC], fp8, tag="ctx")
                # ~20/20/10 split across the two HWDGE rings + SWDGE ring
                eng = (nc.sync, nc.scalar, nc.sync, nc.scalar,
                       nc.gpsimd)[s % 5]
                eng.dma_start(out=ct[:, :], in_=zin[i0:i0 + 3, j0:j0 + 3, :, :])
                return ct

            # critical path first: L1 weights + the first pairs' gathers
            w1v_sb = cpool.tile([72, K * HID], fp8)
            nc.sync.dma_start(out=w1v_sb[:, :], in_=w1v_d[:, :])
            ctx_q = {}
            for s in range(6):
                ctx_q[s] = emit_gather(s)

            # remaining constants (needed a few microseconds in)
            sp_sb = cpool.tile([128, 3], fp32)
            nc.scalar.dma_start(out=sp_sb[:, :], in_=sp_d[:, :])
            b1_sb = sp_sb[:, 0:1]
            ones_ap = sp_sb[0:NC, 1:2]
            tidx_ap = sp_sb[0:NC, 2:3].bitcast(i32)
            w2p_sb = cpool.tile([128, NW2A + NW2B], bf16)
            nc.scalar.dma_start(out=w2p_sb[:, :], in_=w2p_d[:, :])
            w2a_sb = w2p_sb[:, 0:NW2A]
            w2b_sb = w2p_sb[:, NW2A:NW2A + NW2B]

            # all target rows in one indirect gather (partitions 64+ to
            # touch the otherwise-idle high SBUF ports)
            tstage = cpool.tile([128, B_LOC], fp8)
            zflat = zin[:, :, :, :].rearrange("i j k b -> (i j k) b")
            nc.gpsimd.indirect_dma_start(
                out=tstage[64:64 + NC, :],
                out_offset=None,
                in_=zflat,
                in_offset=bass.IndirectOffsetOnAxis(ap=tidx_ap, axis=0),
            )
            t32 = cpool.tile([NC, B_LOC], fp32)
            nc.vector.tensor_copy(out=t32[:, :], in_=tstage[64:64 + NC, :])

            # two L2 accumulators in separate PSUM banks (interleaved
            # accumulation groups cannot share a bank: start=True clears the
            # whole bank's has_written bits)
            logA = pslpool.tile([128, B_LOC], fp32, tag="logA")
            logB = pslpool.tile([128, B_LOC], fp32, tag="logB")

            def emit_l2(pp, hs):
                if pp % 2 == 0:
                    e = pp // 2
                    outsl = logA[0:NA, :]
                    wsl = w2a_sb[:, e * NA:(e + 1) * NA]
                    col = 0
                else:
                    e = (pp - 1) // 2
                    outsl = logB[64:64 + NB, :]
                    wsl = w2b_sb[:, e * NB:(e + 1) * NB]
                    col = 64
                nc.tensor.matmul(
                    outsl, wsl, hs[:, :],
                    start=(pp in (0, 1)),
                    stop=(pp in (NPAIR - 1, NPAIR - 2)),
                    tile_position=(0, col),
                    skip_group_check=True)

            PF = 10  # pairs of gather prefetch
            for s in range(6, 2 * PF):
                ctx_q[s] = emit_gather(s)

            hq = {}
            for p in range(NPAIR):
                sA, sB = 2 * p, 2 * p + 1
                ctA = ctx_q.pop(sA)
                ctB = ctx_q.pop(sB)
                vA = sites[sA][0]
                vB = sites[sB][0]
                h_ps = pspool.tile([128, B_LOC], fp32, tag="hps")
                nc.tensor.matmul(
                    h_ps[0:HID, :], w1v_sb[:, vA * HID:(vA + 1) * HID],
                    ctA[:, :], start=True, stop=True, tile_position=(0, 0))
                nc.tensor.matmul(
                    h_ps[HID:128, :], w1v_sb[:, vB * HID:(vB + 1) * HID],
                    ctB[:, :], start=True, stop=True, tile_position=(0, 64))
                for s in (2 * (p + PF), 2 * (p + PF) + 1):
                    if s < S:
                        ctx_q[s] = emit_gather(s)
                h_sb = hpool.tile([128, B_LOC], bf16, tag="hsb")
                if p % 5 == 4:
                    nc.scalar.activation(
                        out=h_sb[:, :], in_=h_ps[:, :],
                        func=mybir.ActivationFunctionType.Relu,
                        bias=b1_sb[:, 0:1], scale=1.0)
                else:
                    nc.vector.tensor_scalar(
                        out=h_sb[:, :], in0=h_ps[:, :],
                        scalar1=b1_sb[:, 0:1], scalar2=0.0,
                        op0=mybir.AluOpType.add, op1=mybir.AluOpType.max)
                hq[p] = h_sb
                if p % 2 == 1:
                    emit_l2(p - 1, hq.pop(p - 1))
                    emit_l2(p, hq.pop(p))
            for pp in sorted(hq):
                emit_l2(pp, hq.pop(pp))

            # ((logit > -b2) != target) summed over batch, per chain
            junk = cpool.tile([NC, B_LOC], fp32)
            counts = cpool.tile([NC, 1], fp32)
            nc.vector.memset(counts[:, :], 0.0)
            nc.vector.scalar_tensor_tensor(
                out=junk[0:NA, :], in0=logA[0:NA, :], scalar=neg_b2,
                in1=t32[0:NA, :],
                op0=mybir.AluOpType.is_gt, op1=mybir.AluOpType.not_equal,
                accum_out=counts[0:NA, :])
            nc.vector.scalar_tensor_tensor(
                out=junk[32:NC, :], in0=logB[64:64 + NB, :], scalar=neg_b2,
                in1=t32[32:NC, :],
                op0=mybir.AluOpType.is_gt, op1=mybir.AluOpType.not_equal,
                accum_out=counts[32:NC, :])

            tot_ps = pstpool.tile([1, 1], fp32, tag="tot")
            nc.tensor.matmul(tot_ps[:, :], ones_ap, counts[:, :],
                             start=True, stop=True)
            res_sb = cpool.tile([1, 1], fp32)
            nc.vector.tensor_scalar_mul(res_sb[:, :], tot_ps[:, :],
                                        1.0 / float(B * S))
            nc.sync.dma_start(out=outp[:, :], in_=res_sb[:, :])

    nc.compile()
    return nc


def kernel(**inputs):
    global LAST_RESULTS
    from concourse.bass_utils import run_bass_kernel_spmd

    z = np.asarray(inputs["z"], dtype=np.float32)
    in_maps, sites, neg_b2 = _host_prep(
        z, np.asarray(inputs["W1"], dtype=np.float32),
        np.asarray(inputs["b1"], dtype=np.float32),
        np.asarray(inputs["W2"], dtype=np.float32),
        inputs["b2"], inputs["b_idx"], inputs["i_idx"], inputs["j_idx"])

    nc = _build_program(sites, neg_b2)

    res = run_bass_kernel_spmd(nc, in_maps, list(range(N_CORES)))
    LAST_RESULTS = res
    total = np.float32(0.0)
    for r in res.results:
        total += np.float32(r["out"][0, 0])
    return np.float32(total)


# revision 4
# speedup vs baseline: 1.2232x; 1.0268x over previous
"""Trainium2 Bass kernel for nn_LocalEnergyCore — v2.

Differences vs baseline:
  - z shards stored [H+2, W+2, K, B_loc] in fp8e4m3 (bits 0/1 are exact in
    fp8): gather DMA bytes halve, descriptors stay at the 512B line-rate
    minimum.
  - ONE gather DMA per site ([3,3,8,512] block -> [72, 512] ctx tile,
    partition order c = di*24 + dj*8 + k) instead of 3.
  - The 50 target rows come via a single SWDGE indirect_dma_start driven by
    an int32 row-index table (one instruction, 50 descriptors).
  - L1 pair matmuls carry explicit tile_position (0,0)/(0,64) so the two
    sites of a pair run concurrently on the PE's column halves.
  - relu+bias alternates between DVE (tensor_scalar add+max, per-partition
    bias AP) and ACT (activation Relu with bias) so neither engine is the
    bottleneck.
  - L2 collect is split into TWO accumulation chains in different PSUM banks
    (even pairs -> bank A rows 0:26 at col tile 0, odd pairs -> bank B rows
    64:88 at col tile 64); chains run concurrently on the PE column halves.
    Each accumulator holds a disjoint contiguous set of sites' logits, so the
    compare+count runs once per accumulator (a PSUM in0 + SBUF target in1)
    with no cross-bank add.
"""

import sys

for _p in ("/opt/trn_rl_repo",):
    if _p not in sys.path:
        sys.path.insert(0, _p)

import numpy as np
import ml_dtypes

B, K, H, W = 4096, 8, 64, 64
S, HID, CTX = 50, 64, 71
N_CORES = 8
B_LOC = B // N_CORES
NPAIR = S // 2

BF16 = ml_dtypes.bfloat16
FP8 = ml_dtypes.float8_e4m3fn

LAST_RESULTS = None  # test harness introspection


def _host_prep(z, W1, b1, W2, b2, b_idx, i_idx, j_idx):
    """Shard + lay out inputs; returns (in_maps, site list, -b2)."""
    b_idx = np.asarray(b_idx).astype(np.int64)
    i_idx = np.asarray(i_idx).astype(np.int64)
    j_idx = np.asarray(j_idx).astype(np.int64)
    sites = [(int(b_idx[s]), int(i_idx[s]), int(j_idx[s])) for s in range(S)]

    # variant W1 matrices: [72, HID] with row t = W1[t - (t > drop)], row
    # drop = 0. Rows permuted to the gather order c = di*24 + dj*8 + k
    # (original order is position-major: c0 = (3*di+dj)*8 + k).
    W1V = np.zeros((K, 72, HID), dtype=np.float32)
    t = np.arange(72)
    for v in range(K):
        drop = 4 * K + v
        src = t - (t > drop)
        W1V[v] = W1[np.minimum(src, CTX - 1)]
        W1V[v, drop] = 0.0
    perm = np.empty(72, dtype=np.int64)
    for di in range(3):
        for dj in range(3):
            for k in range(K):
                perm[di * 24 + dj * 8 + k] = (3 * di + dj) * K + k
    W1V = W1V[:, perm, :]
    w1v_np = np.ascontiguousarray(
        W1V.transpose(1, 0, 2).reshape(72, K * HID)).astype(FP8)

    # block-diagonal W2 for the two accumulated L2 chains. Even pairs
    # (p=0,2,..,24) collect into chain A compare-rows 0:26; odd pairs into
    # chain B compare-rows 32:56 (engine partition bases must be 32-aligned).
    # cmp_row[s] = site s's row in the packed [56] compare layout.
    NA = 2 * ((NPAIR + 1) // 2)       # 26 chain-A rows
    NB = S - NA                       # 24 chain-B rows
    NC = 32 + NB                      # compare layout height (56)
    cmp_row = np.zeros(S, dtype=np.int64)
    w2a = np.zeros((128, (NPAIR + 1) // 2, NA), dtype=np.float32)
    w2b = np.zeros((128, NPAIR // 2, NB), dtype=np.float32)
    for p in range(NPAIR):
        for q in (0, 1):
            s = 2 * p + q
            if p % 2 == 0:
                loc = 2 * (p // 2) + q
                w2a[q * HID:(q + 1) * HID, p // 2, loc] = W2
                cmp_row[s] = loc
            else:
                loc = 2 * ((p - 1) // 2) + q
                w2b[q * HID:(q + 1) * HID, (p - 1) // 2, loc] = W2
                cmp_row[s] = 32 + loc
    w2a_np = np.ascontiguousarray(
        w2a.reshape(128, ((NPAIR + 1) // 2) * NA)).astype(BF16)
    w2b_np = np.ascontiguousarray(
        w2b.reshape(128, (NPAIR // 2) * NB)).astype(BF16)

    # pack w2a|w2b into one DMA payload
    w2pack = np.concatenate([w2a_np, w2b_np], axis=1)

    # target row index per compare-row into z flattened [(i, j, k), b];
    # onesvec masks the unused rows 26:32 out of the final reduction.
    # Pack bias1 / onesvec / tidx(bit-cast) into one [128, 3] f32 payload.
    tidx = np.zeros((NC, 1), dtype=np.int32)
    onesvec = np.zeros((NC, 1), dtype=np.float32)
    for s, (v, i0, j0) in enumerate(sites):
        tidx[cmp_row[s], 0] = ((1 + i0) * 66 + (1 + j0)) * K + v
        onesvec[cmp_row[s], 0] = 1.0
    smallpack = np.zeros((128, 3), dtype=np.float32)
    smallpack[:, 0] = np.concatenate([b1, b1]).astype(np.float32)
    smallpack[0:NC, 1] = onesvec[:, 0]
    smallpack[0:NC, 2] = tidx[:, 0].view(np.float32)

    # z shards: [H+2, W+2, K, B_loc] fp8 with wrap halo, batch innermost
    in_maps = []
    z8 = z.astype(FP8)
    for c in range(N_CORES):
        zt = z8[c * B_LOC:(c + 1) * B_LOC].transpose(2, 3, 1, 0)  # [H,W,K,Bl]
        zp = np.empty((H + 2, W + 2, K, B_LOC), dtype=FP8)
        zp[1:H + 1, 1:W + 1] = zt
        zp[0, 1:W + 1] = zt[H - 1]
        zp[H + 1, 1:W + 1] = zt[0]
        zp[:, 0] = zp[:, W]
        zp[:, W + 1] = zp[:, 1]
        in_maps.append({
            "z": np.ascontiguousarray(zp),
            "w1v": w1v_np,
            "w2pack": w2pack,
            "smallpack": smallpack,
        })
    return in_maps, sites, -float(np.asarray(b2))


def _build_program(sites, neg_b2):
    """Emit the per-core Bass program (identical across cores)."""
    import concourse.bacc as bacc
    import concourse.bass as bass
    import concourse.mybir as mybir
    import concourse.tile as tile

    fp32 = mybir.dt.float32
    bf16 = mybir.dt.bfloat16
    fp8 = mybir.dt.float8e4
    i32 = mybir.dt.int32

    nc = bacc.Bacc("TRN2", target_bir_lowering=False, debug=False,
                   num_devices=N_CORES)

    NA = 2 * ((NPAIR + 1) // 2)       # 26 chain-A rows
    NB = S - NA                       # 24 chain-B rows
    NC = 32 + NB                      # compare layout height (56)

    NW2A = ((NPAIR + 1) // 2) * NA
    NW2B = (NPAIR // 2) * NB

    zin = nc.dram_tensor("z", [H + 2, W + 2, K, B_LOC], fp8,
                         kind="ExternalInput")
    w1v_d = nc.dram_tensor("w1v", [72, K * HID], fp8, kind="ExternalInput")
    w2p_d = nc.dram_tensor("w2pack", [128, NW2A + NW2B], bf16,
                           kind="ExternalInput")
    sp_d = nc.dram_tensor("smallpack", [128, 3], fp32, kind="ExternalInput")
    outp = nc.dram_tensor("out", [1, 1], fp32, kind="ExternalOutput")

    with tile.TileContext(nc) as tc:
        with (
            tc.tile_pool(name="const", bufs=1) as cpool,
            tc.tile_pool(name="ctx", bufs=24) as ctxpool,
            tc.tile_pool(name="hsb", bufs=8) as hpool,
            tc.tile_pool(name="ps", bufs=4, space="PSUM") as pspool,
            tc.tile_pool(name="psl", bufs=1, space="PSUM") as pslpool,
            tc.tile_pool(name="pst", bufs=1, space="PSUM") as pstpool,
        ):
            def emit_gather(s):
                v, i0, j0 = sites[s]
                ct = ctxpool.tile([72, B_LOC], fp8, tag="ctx")
                # ~17/17/16 split across the two HWDGE rings + SWDGE ring
                eng = (nc.sync, nc.scalar, nc.gpsimd)[s % 3]
                eng.dma_start(out=ct[:, :], in_=zin[i0:i0 + 3, j0:j0 + 3, :, :])
                return ct

            # critical path first: L1 weights + the first pairs' gathers
            w1v_sb = cpool.tile([72, K * HID], fp8)
            nc.sync.dma_start(out=w1v_sb[:, :], in_=w1v_d[:, :])
            ctx_q = {}
            for s in range(6):
                ctx_q[s] = emit_gather(s)

            # remaining constants (needed a few microseconds in)
            sp_sb = cpool.tile([128, 3], fp32)
            nc.scalar.dma_start(out=sp_sb[:, :], in_=sp_d[:, :])
            b1_sb = sp_sb[:, 0:1]
            ones_ap = sp_sb[0:NC, 1:2]
            tidx_ap = sp_sb[0:NC, 2:3].bitcast(i32)
            w2p_sb = cpool.tile([128, NW2A + NW2B], bf16)
            nc.scalar.dma_start(out=w2p_sb[:, :], in_=w2p_d[:, :])
            w2a_sb = w2p_sb[:, 0:NW2A]
            w2b_sb = w2p_sb[:, NW2A:NW2A + NW2B]

            # all target rows in one indirect gather (partitions 64+ to
            # touch the otherwise-idle high SBUF ports)
            tstage = cpool.tile([128, B_LOC], fp8)
            zflat = zin[:, :, :, :].rearrange("i j k b -> (i j k) b")
            nc.gpsimd.indirect_dma_start(
                out=tstage[64:64 + NC, :],
                out_offset=None,
                in_=zflat,
                in_offset=bass.IndirectOffsetOnAxis(ap=tidx_ap, axis=0),
            )
            t32 = cpool.tile([NC, B_LOC], fp32)
            nc.vector.tensor_copy(out=t32[:, :], in_=tstage[64:64 + NC, :])

            # two L2 accumulators in separate PSUM banks (interleaved
            # accumulation groups cannot share a bank: start=True clears the
            # whole bank's has_written bits)
            logA = pslpool.tile([128, B_LOC], fp32, tag="logA")
            logB = pslpool.tile([128, B_LOC], fp32, tag="logB")

            def emit_l2(pp, hs):
                if pp % 2 == 0:
                    e = pp // 2
                    outsl = logA[0:NA, :]
                    wsl = w2a_sb[:, e * NA:(e + 1) * NA]
                    col = 0
                else:
                    e = (pp - 1) // 2
                    outsl = logB[64:64 + NB, :]
                    wsl = w2b_sb[:, e * NB:(e + 1) * NB]
                    col = 64
                nc.tensor.matmul(
                    outsl, wsl, hs[:, :],
                    start=(pp in (0, 1)),
                    stop=(pp in (NPAIR - 1, NPAIR - 2)),
                    tile_position=(0, col),
                    skip_group_check=True)

            PF = 10  # pairs of gather prefetch
            for s in range(6, 2 * PF):
                ctx_q[s] = emit_gather(s)

            hq = {}
            for p in range(NPAIR):
                sA, sB = 2 * p, 2 * p + 1
                ctA = ctx_q.pop(sA)
                ctB = ctx_q.pop(sB)
                vA = sites[sA][0]
                vB = sites[sB][0]
                h_ps = pspool.tile([128, B_LOC], fp32, tag="hps")
                nc.tensor.matmul(
                    h_ps[0:HID, :], w1v_sb[:, vA * HID:(vA + 1) * HID],
                    ctA[:, :], start=True, stop=True, tile_position=(0, 0))
                nc.tensor.matmul(
                    h_ps[HID:128, :], w1v_sb[:, vB * HID:(vB + 1) * HID],
                    ctB[:, :], start=True, stop=True, tile_position=(0, 64))
                for s in (2 * (p + PF), 2 * (p + PF) + 1):
                    if s < S:
                        ctx_q[s] = emit_gather(s)
                h_sb = hpool.tile([128, B_LOC], bf16, tag="hsb")
                if p % 5 == 4:
                    nc.scalar.activation(
                        out=h_sb[:, :], in_=h_ps[:, :],
                        func=mybir.ActivationFunctionType.Relu,
                        bias=b1_sb[:, 0:1], scale=1.0)
                else:
                    nc.vector.tensor_scalar(
                        out=h_sb[:, :], in0=h_ps[:, :],
                        scalar1=b1_sb[:, 0:1], scalar2=0.0,
                        op0=mybir.AluOpType.add, op1=mybir.AluOpType.max)
                hq[p] = h_sb
                if p % 2 == 1:
                    emit_l2(p - 1, hq.pop(p - 1))
                    emit_l2(p, hq.pop(p))
            for pp in sorted(hq):
                emit_l2(pp, hq.pop(pp))

            # ((logit > -b2) != target) summed over batch, per chain
            junk = cpool.tile([NC, B_LOC], fp32)
            counts = cpool.tile([NC, 1], fp32)
            nc.vector.memset(counts[:, :], 0.0)
            nc.vector.scalar_tensor_tensor(
                out=junk[0:NA, :], in0=logA[0:NA, :], scalar=neg_b2,
                in1=t32[0:NA, :],
                op0=mybir.AluOpType.is_gt, op1=mybir.AluOpType.not_equal,
                accum_out=counts[0:NA, :])
            nc.vector.scalar_tensor_tensor(
                out=junk[32:NC, :], in0=logB[64:64 + NB, :], scalar=neg_b2,
                in1=t32[32:NC, :],
                op0=mybir.AluOpType.is_gt, op1=mybir.AluOpType.not_equal,
                accum_out=counts[32:NC, :])

            tot_ps = pstpool.tile([1, 1], fp32, tag="tot")
            nc.tensor.matmul(tot_ps[:, :], ones_ap, counts[:, :],
                             start=True, stop=True)
            res_sb = cpool.tile([1, 1], fp32)
            nc.vector.tensor_scalar_mul(res_sb[:, :], tot_ps[:, :],
                                        1.0 / float(B * S))
            nc.sync.dma_start(out=outp[:, :], in_=res_sb[:, :])

    nc.compile()
    return nc


def kernel(**inputs):
    global LAST_RESULTS
    from concourse.bass_utils import run_bass_kernel_spmd

    z = np.asarray(inputs["z"], dtype=np.float32)
    in_maps, sites, neg_b2 = _host_prep(
        z, np.asarray(inputs["W1"], dtype=np.float32),
        np.asarray(inputs["b1"], dtype=np.float32),
        np.asarray(inputs["W2"], dtype=np.float32),
        inputs["b2"], inputs["b_idx"], inputs["i_idx"], inputs["j_idx"])

    nc = _build_program(sites, neg_b2)

    res = run_bass_kernel_spmd(nc, in_maps, list(range(N_CORES)))
    LAST_RESULTS = res
    total = np.float32(0.0)
    for r in res.results:
        total += np.float32(r["out"][0, 0])
    return np.float32(total)


# revision 5
# speedup vs baseline: 1.2546x; 1.0256x over previous
"""Trainium2 Bass kernel for nn_LocalEnergyCore — v2.

Differences vs baseline:
  - z shards stored [H+2, W+2, K, B_loc] in fp8e4m3 (bits 0/1 are exact in
    fp8): gather DMA bytes halve, descriptors stay at the 512B line-rate
    minimum.
  - ONE gather DMA per site ([3,3,8,512] block -> [72, 512] ctx tile,
    partition order c = di*24 + dj*8 + k) instead of 3.
  - The 50 target rows come via a single SWDGE indirect_dma_start driven by
    an int32 row-index table (one instruction, 50 descriptors).
  - L1 pair matmuls carry explicit tile_position (0,0)/(0,64) so the two
    sites of a pair run concurrently on the PE's column halves.
  - relu+bias alternates between DVE (tensor_scalar add+max, per-partition
    bias AP) and ACT (activation Relu with bias) so neither engine is the
    bottleneck.
  - L2 collect is split into TWO accumulation chains in different PSUM banks
    (even pairs -> bank A rows 0:26 at col tile 0, odd pairs -> bank B rows
    64:88 at col tile 64); chains run concurrently on the PE column halves.
    Each accumulator holds a disjoint contiguous set of sites' logits, so the
    compare+count runs once per accumulator (a PSUM in0 + SBUF target in1)
    with no cross-bank add.
"""

import sys

for _p in ("/opt/trn_rl_repo",):
    if _p not in sys.path:
        sys.path.insert(0, _p)

import numpy as np
import ml_dtypes

B, K, H, W = 4096, 8, 64, 64
S, HID, CTX = 50, 64, 71
N_CORES = 8
B_LOC = B // N_CORES
NPAIR = S // 2

BF16 = ml_dtypes.bfloat16
FP8 = ml_dtypes.float8_e4m3fn

LAST_RESULTS = None  # test harness introspection


def _host_prep(z, W1, b1, W2, b2, b_idx, i_idx, j_idx):
    """Shard + lay out inputs; returns (in_maps, site list, -b2)."""
    b_idx = np.asarray(b_idx).astype(np.int64)
    i_idx = np.asarray(i_idx).astype(np.int64)
    j_idx = np.asarray(j_idx).astype(np.int64)
    sites = [(int(b_idx[s]), int(i_idx[s]), int(j_idx[s])) for s in range(S)]

    # variant W1 matrices: [72, HID] with row t = W1[t - (t > drop)], row
    # drop = 0. Rows permuted to the gather order c = di*24 + dj*8 + k
    # (original order is position-major: c0 = (3*di+dj)*8 + k).
    W1V = np.zeros((K, 72, HID), dtype=np.float32)
    t = np.arange(72)
    for v in range(K):
        drop = 4 * K + v
        src = t - (t > drop)
        W1V[v] = W1[np.minimum(src, CTX - 1)]
        W1V[v, drop] = 0.0
    perm = np.empty(72, dtype=np.int64)
    for di in range(3):
        for dj in range(3):
            for k in range(K):
                perm[di * 24 + dj * 8 + k] = (3 * di + dj) * K + k
    W1V = W1V[:, perm, :]
    w1v_np = np.ascontiguousarray(
        W1V.transpose(1, 0, 2).reshape(72, K * HID)).astype(FP8)

    # block-diagonal W2 for the two accumulated L2 chains. Even pairs
    # (p=0,2,..,24) collect into chain A compare-rows 0:26; odd pairs into
    # chain B compare-rows 32:56 (engine partition bases must be 32-aligned).
    # cmp_row[s] = site s's row in the packed [56] compare layout.
    NA = 2 * ((NPAIR + 1) // 2)       # 26 chain-A rows
    NB = S - NA                       # 24 chain-B rows
    NC = 32 + NB                      # compare layout height (56)
    cmp_row = np.zeros(S, dtype=np.int64)
    w2a = np.zeros((128, (NPAIR + 1) // 2, NA), dtype=np.float32)
    w2b = np.zeros((128, NPAIR // 2, NB), dtype=np.float32)
    for p in range(NPAIR):
        for q in (0, 1):
            s = 2 * p + q
            if p % 2 == 0:
                loc = 2 * (p // 2) + q
                w2a[q * HID:(q + 1) * HID, p // 2, loc] = W2
                cmp_row[s] = loc
            else:
                loc = 2 * ((p - 1) // 2) + q
                w2b[q * HID:(q + 1) * HID, (p - 1) // 2, loc] = W2
                cmp_row[s] = 32 + loc
    w2a_np = np.ascontiguousarray(
        w2a.reshape(128, ((NPAIR + 1) // 2) * NA)).astype(BF16)
    w2b_np = np.ascontiguousarray(
        w2b.reshape(128, (NPAIR // 2) * NB)).astype(BF16)

    # pack w2a|w2b into one DMA payload
    w2pack = np.concatenate([w2a_np, w2b_np], axis=1)

    # target row index per compare-row into z flattened [(i, j, k), b];
    # onesvec masks the unused rows 26:32 out of the final reduction.
    # Pack bias1 / onesvec / tidx(bit-cast) into one [128, 3] f32 payload.
    tidx = np.zeros((NC, 1), dtype=np.int32)
    onesvec = np.zeros((NC, 1), dtype=np.float32)
    for s, (v, i0, j0) in enumerate(sites):
        tidx[cmp_row[s], 0] = ((1 + i0) * 66 + (1 + j0)) * K + v
        onesvec[cmp_row[s], 0] = 1.0
    smallpack = np.zeros((128, 3), dtype=np.float32)
    smallpack[:, 0] = np.concatenate([b1, b1]).astype(np.float32)
    smallpack[0:NC, 1] = onesvec[:, 0]
    smallpack[0:NC, 2] = tidx[:, 0].view(np.float32)

    # z shards: [H+2, W+2, K, B_loc] fp8 with wrap halo, batch innermost
    in_maps = []
    z8 = z.astype(FP8)
    for c in range(N_CORES):
        zt = z8[c * B_LOC:(c + 1) * B_LOC].transpose(2, 3, 1, 0)  # [H,W,K,Bl]
        zp = np.empty((H + 2, W + 2, K, B_LOC), dtype=FP8)
        zp[1:H + 1, 1:W + 1] = zt
        zp[0, 1:W + 1] = zt[H - 1]
        zp[H + 1, 1:W + 1] = zt[0]
        zp[:, 0] = zp[:, W]
        zp[:, W + 1] = zp[:, 1]
        in_maps.append({
            "z": np.ascontiguousarray(zp),
            "w1v": w1v_np,
            "w2pack": w2pack,
            "smallpack": smallpack,
        })
    return in_maps, sites, -float(np.asarray(b2))


def _build_program(sites, neg_b2):
    """Emit the per-core Bass program (identical across cores)."""
    import concourse.bacc as bacc
    import concourse.bass as bass
    import concourse.mybir as mybir
    import concourse.tile as tile

    fp32 = mybir.dt.float32
    bf16 = mybir.dt.bfloat16
    fp8 = mybir.dt.float8e4
    i32 = mybir.dt.int32

    nc = bacc.Bacc("TRN2", target_bir_lowering=False, debug=False,
                   num_devices=N_CORES)

    NA = 2 * ((NPAIR + 1) // 2)       # 26 chain-A rows
    NB = S - NA                       # 24 chain-B rows
    NC = 32 + NB                      # compare layout height (56)

    NW2A = ((NPAIR + 1) // 2) * NA
    NW2B = (NPAIR // 2) * NB

    zin = nc.dram_tensor("z", [H + 2, W + 2, K, B_LOC], fp8,
                         kind="ExternalInput")
    w1v_d = nc.dram_tensor("w1v", [72, K * HID], fp8, kind="ExternalInput")
    w2p_d = nc.dram_tensor("w2pack", [128, NW2A + NW2B], bf16,
                           kind="ExternalInput")
    sp_d = nc.dram_tensor("smallpack", [128, 3], fp32, kind="ExternalInput")
    outp = nc.dram_tensor("out", [1, 1], fp32, kind="ExternalOutput")

    with tile.TileContext(nc) as tc:
        with (
            tc.tile_pool(name="const", bufs=1) as cpool,
            tc.tile_pool(name="ctx", bufs=24) as ctxpool,
            tc.tile_pool(name="hsb", bufs=8) as hpool,
            tc.tile_pool(name="ps", bufs=4, space="PSUM") as pspool,
            tc.tile_pool(name="psl", bufs=1, space="PSUM") as pslpool,
            tc.tile_pool(name="pst", bufs=1, space="PSUM") as pstpool,
        ):
            def emit_gather(s):
                v, i0, j0 = sites[s]
                ct = ctxpool.tile([72, B_LOC], fp8, tag="ctx")
                # ~15/17/18 split across the two HWDGE rings + SWDGE ring
                if s % 3 == 2 or s % 10 == 9:
                    eng = nc.gpsimd
                elif s % 3 == 0:
                    eng = nc.sync
                else:
                    eng = nc.scalar
                eng.dma_start(out=ct[:, :], in_=zin[i0:i0 + 3, j0:j0 + 3, :, :])
                return ct

            # critical path first: L1 weights + the first pairs' gathers
            w1v_sb = cpool.tile([72, K * HID], fp8)
            nc.sync.dma_start(out=w1v_sb[:, :], in_=w1v_d[:, :])
            ctx_q = {}
            for s in range(6):
                ctx_q[s] = emit_gather(s)

            # remaining constants (needed a few microseconds in)
            sp_sb = cpool.tile([128, 3], fp32)
            nc.scalar.dma_start(out=sp_sb[:, :], in_=sp_d[:, :])
            b1_sb = sp_sb[:, 0:1]
            ones_ap = sp_sb[0:NC, 1:2]
            tidx_ap = sp_sb[0:NC, 2:3].bitcast(i32)
            w2p_sb = cpool.tile([128, NW2A + NW2B], bf16)
            nc.scalar.dma_start(out=w2p_sb[:, :], in_=w2p_d[:, :])
            w2a_sb = w2p_sb[:, 0:NW2A]
            w2b_sb = w2p_sb[:, NW2A:NW2A + NW2B]

            # all target rows in one indirect gather (partitions 64+ to
            # touch the otherwise-idle high SBUF ports)
            tstage = cpool.tile([128, B_LOC], fp8)
            zflat = zin[:, :, :, :].rearrange("i j k b -> (i j k) b")
            nc.gpsimd.indirect_dma_start(
                out=tstage[64:64 + NC, :],
                out_offset=None,
                in_=zflat,
                in_offset=bass.IndirectOffsetOnAxis(ap=tidx_ap, axis=0),
            )
            t32 = cpool.tile([NC, B_LOC], fp32)
            nc.vector.tensor_copy(out=t32[:, :], in_=tstage[64:64 + NC, :])

            # two L2 accumulators in separate PSUM banks (interleaved
            # accumulation groups cannot share a bank: start=True clears the
            # whole bank's has_written bits)
            logA = pslpool.tile([128, B_LOC], fp32, tag="logA")
            logB = pslpool.tile([128, B_LOC], fp32, tag="logB")

            def emit_l2(pp, hs):
                if pp % 2 == 0:
                    e = pp // 2
                    outsl = logA[0:NA, :]
                    wsl = w2a_sb[:, e * NA:(e + 1) * NA]
                    col = 0
                else:
                    e = (pp - 1) // 2
                    outsl = logB[64:64 + NB, :]
                    wsl = w2b_sb[:, e * NB:(e + 1) * NB]
                    col = 64
                nc.tensor.matmul(
                    outsl, wsl, hs[:, :],
                    start=(pp in (0, 1)),
                    stop=(pp in (NPAIR - 1, NPAIR - 2)),
                    tile_position=(0, col),
                    skip_group_check=True)

            PF = 10  # pairs of gather prefetch
            for s in range(6, 2 * PF):
                ctx_q[s] = emit_gather(s)

            hq = {}
            for p in range(NPAIR):
                sA, sB = 2 * p, 2 * p + 1
                ctA = ctx_q.pop(sA)
                ctB = ctx_q.pop(sB)
                vA = sites[sA][0]
                vB = sites[sB][0]
                h_ps = pspool.tile([128, B_LOC], fp32, tag="hps")
                nc.tensor.matmul(
                    h_ps[0:HID, :], w1v_sb[:, vA * HID:(vA + 1) * HID],
                    ctA[:, :], start=True, stop=True, tile_position=(0, 0))
                nc.tensor.matmul(
                    h_ps[HID:128, :], w1v_sb[:, vB * HID:(vB + 1) * HID],
                    ctB[:, :], start=True, stop=True, tile_position=(0, 64))
                for s in (2 * (p + PF), 2 * (p + PF) + 1):
                    if s < S:
                        ctx_q[s] = emit_gather(s)
                h_sb = hpool.tile([128, B_LOC], bf16, tag="hsb")
                if p % 5 == 4:
                    nc.scalar.activation(
                        out=h_sb[:, :], in_=h_ps[:, :],
                        func=mybir.ActivationFunctionType.Relu,
                        bias=b1_sb[:, 0:1], scale=1.0)
                else:
                    nc.vector.tensor_scalar(
                        out=h_sb[:, :], in0=h_ps[:, :],
                        scalar1=b1_sb[:, 0:1], scalar2=0.0,
                        op0=mybir.AluOpType.add, op1=mybir.AluOpType.max)
                hq[p] = h_sb
                if p % 2 == 1:
                    emit_l2(p - 1, hq.pop(p - 1))
                    emit_l2(p, hq.pop(p))
            for pp in sorted(hq):
                emit_l2(pp, hq.pop(pp))

            # ((logit > -b2) != target) summed over batch, per chain
            junk = cpool.tile([NC, B_LOC], fp32)
            counts = cpool.tile([NC, 1], fp32)
            nc.vector.memset(counts[:, :], 0.0)
            nc.vector.scalar_tensor_tensor(
                out=junk[0:NA, :], in0=logA[0:NA, :], scalar=neg_b2,
                in1=t32[0:NA, :],
                op0=mybir.AluOpType.is_gt, op1=mybir.AluOpType.not_equal,
                accum_out=counts[0:NA, :])
            nc.vector.scalar_tensor_tensor(
                out=junk[32:NC, :], in0=logB[64:64 + NB, :], scalar=neg_b2,
                in1=t32[32:NC, :],
                op0=mybir.AluOpType.is_gt, op1=mybir.AluOpType.not_equal,
                accum_out=counts[32:NC, :])

            tot_ps = pstpool.tile([1, 1], fp32, tag="tot")
            nc.tensor.matmul(tot_ps[:, :], ones_ap, counts[:, :],
                             start=True, stop=True)
            res_sb = cpool.tile([1, 1], fp32)
            nc.vector.tensor_scalar_mul(res_sb[:, :], tot_ps[:, :],
                                        1.0 / float(B * S))
            nc.sync.dma_start(out=outp[:, :], in_=res_sb[:, :])

    nc.compile()
    return nc


def kernel(**inputs):
    global LAST_RESULTS
    from concourse.bass_utils import run_bass_kernel_spmd

    z = np.asarray(inputs["z"], dtype=np.float32)
    in_maps, sites, neg_b2 = _host_prep(
        z, np.asarray(inputs["W1"], dtype=np.float32),
        np.asarray(inputs["b1"], dtype=np.float32),
        np.asarray(inputs["W2"], dtype=np.float32),
        inputs["b2"], inputs["b_idx"], inputs["i_idx"], inputs["j_idx"])

    nc = _build_program(sites, neg_b2)

    res = run_bass_kernel_spmd(nc, in_maps, list(range(N_CORES)))
    LAST_RESULTS = res
    total = np.float32(0.0)
    for r in res.results:
        total += np.float32(r["out"][0, 0])
    return np.float32(total)


# revision 6
# speedup vs baseline: 1.2687x; 1.0112x over previous
"""Trainium2 Bass kernel for nn_LocalEnergyCore — v2.

Differences vs baseline:
  - z shards stored [H+2, W+2, K, B_loc] in fp8e4m3 (bits 0/1 are exact in
    fp8): gather DMA bytes halve, descriptors stay at the 512B line-rate
    minimum.
  - ONE gather DMA per site ([3,3,8,512] block -> [72, 512] ctx tile,
    partition order c = di*24 + dj*8 + k) instead of 3.
  - The 50 target rows come via a single SWDGE indirect_dma_start driven by
    an int32 row-index table (one instruction, 50 descriptors).
  - L1 pair matmuls carry explicit tile_position (0,0)/(0,64) so the two
    sites of a pair run concurrently on the PE's column halves.
  - relu+bias alternates between DVE (tensor_scalar add+max, per-partition
    bias AP) and ACT (activation Relu with bias) so neither engine is the
    bottleneck.
  - L2 collect is split into TWO accumulation chains in different PSUM banks
    (even pairs -> bank A rows 0:26 at col tile 0, odd pairs -> bank B rows
    64:88 at col tile 64); chains run concurrently on the PE column halves.
    Each accumulator holds a disjoint contiguous set of sites' logits, so the
    compare+count runs once per accumulator (a PSUM in0 + SBUF target in1)
    with no cross-bank add.
"""

import sys

for _p in ("/opt/trn_rl_repo",):
    if _p not in sys.path:
        sys.path.insert(0, _p)

import numpy as np
import ml_dtypes

B, K, H, W = 4096, 8, 64, 64
S, HID, CTX = 50, 64, 71
N_CORES = 8
B_LOC = B // N_CORES
NPAIR = S // 2

BF16 = ml_dtypes.bfloat16
FP8 = ml_dtypes.float8_e4m3fn

LAST_RESULTS = None  # test harness introspection


def _host_prep(z, W1, b1, W2, b2, b_idx, i_idx, j_idx):
    """Shard + lay out inputs; returns (in_maps, site list, -b2)."""
    b_idx = np.asarray(b_idx).astype(np.int64)
    i_idx = np.asarray(i_idx).astype(np.int64)
    j_idx = np.asarray(j_idx).astype(np.int64)
    sites = [(int(b_idx[s]), int(i_idx[s]), int(j_idx[s])) for s in range(S)]

    # variant W1 matrices: [72, HID] with row t = W1[t - (t > drop)], row
    # drop = 0. Rows permuted to the gather order c = di*24 + dj*8 + k
    # (original order is position-major: c0 = (3*di+dj)*8 + k).
    W1V = np.zeros((K, 72, HID), dtype=np.float32)
    t = np.arange(72)
    for v in range(K):
        drop = 4 * K + v
        src = t - (t > drop)
        W1V[v] = W1[np.minimum(src, CTX - 1)]
        W1V[v, drop] = 0.0
    perm = np.empty(72, dtype=np.int64)
    for di in range(3):
        for dj in range(3):
            for k in range(K):
                perm[di * 24 + dj * 8 + k] = (3 * di + dj) * K + k
    W1V = W1V[:, perm, :]
    w1v_np = np.ascontiguousarray(
        W1V.transpose(1, 0, 2).reshape(72, K * HID)).astype(FP8)

    # block-diagonal W2 for the two accumulated L2 chains. Even pairs
    # (p=0,2,..,24) collect into chain A compare-rows 0:26; odd pairs into
    # chain B compare-rows 32:56 (engine partition bases must be 32-aligned).
    # cmp_row[s] = site s's row in the packed [56] compare layout.
    NA = 2 * ((NPAIR + 1) // 2)       # 26 chain-A rows
    NB = S - NA                       # 24 chain-B rows
    NC = 32 + NB                      # compare layout height (56)
    cmp_row = np.zeros(S, dtype=np.int64)
    w2a = np.zeros((128, (NPAIR + 1) // 2, NA), dtype=np.float32)
    w2b = np.zeros((128, NPAIR // 2, NB), dtype=np.float32)
    for p in range(NPAIR):
        for q in (0, 1):
            s = 2 * p + q
            if p % 2 == 0:
                loc = 2 * (p // 2) + q
                w2a[q * HID:(q + 1) * HID, p // 2, loc] = W2
                cmp_row[s] = loc
            else:
                loc = 2 * ((p - 1) // 2) + q
                w2b[q * HID:(q + 1) * HID, (p - 1) // 2, loc] = W2
                cmp_row[s] = 32 + loc
    w2a_np = np.ascontiguousarray(
        w2a.reshape(128, ((NPAIR + 1) // 2) * NA)).astype(BF16)
    w2b_np = np.ascontiguousarray(
        w2b.reshape(128, (NPAIR // 2) * NB)).astype(BF16)

    # pack w2a|w2b into one DMA payload
    w2pack = np.concatenate([w2a_np, w2b_np], axis=1)

    # target row index per compare-row into z flattened [(i, j, k), b];
    # onesvec masks the unused rows 26:32 out of the final reduction.
    # Pack bias1 / onesvec / tidx(bit-cast) into one [128, 3] f32 payload.
    tidx = np.zeros((NC, 1), dtype=np.int32)
    onesvec = np.zeros((NC, 1), dtype=np.float32)
    for s, (v, i0, j0) in enumerate(sites):
        tidx[cmp_row[s], 0] = ((1 + i0) * 66 + (1 + j0)) * K + v
        onesvec[cmp_row[s], 0] = 1.0
    smallpack = np.zeros((128, 3), dtype=np.float32)
    smallpack[:, 0] = np.concatenate([b1, b1]).astype(np.float32)
    smallpack[0:NC, 1] = onesvec[:, 0]
    smallpack[0:NC, 2] = tidx[:, 0].view(np.float32)

    # z shards: [H+2, W+2, K, B_loc] fp8 with wrap halo, batch innermost
    in_maps = []
    z8 = z.astype(FP8)
    for c in range(N_CORES):
        zt = z8[c * B_LOC:(c + 1) * B_LOC].transpose(2, 3, 1, 0)  # [H,W,K,Bl]
        zp = np.empty((H + 2, W + 2, K, B_LOC), dtype=FP8)
        zp[1:H + 1, 1:W + 1] = zt
        zp[0, 1:W + 1] = zt[H - 1]
        zp[H + 1, 1:W + 1] = zt[0]
        zp[:, 0] = zp[:, W]
        zp[:, W + 1] = zp[:, 1]
        in_maps.append({
            "z": np.ascontiguousarray(zp),
            "w1v": w1v_np,
            "w2pack": w2pack,
            "smallpack": smallpack,
        })
    return in_maps, sites, -float(np.asarray(b2))


def _build_program(sites, neg_b2):
    """Emit the per-core Bass program (identical across cores)."""
    import concourse.bacc as bacc
    import concourse.bass as bass
    import concourse.mybir as mybir
    import concourse.tile as tile

    fp32 = mybir.dt.float32
    bf16 = mybir.dt.bfloat16
    fp8 = mybir.dt.float8e4
    i32 = mybir.dt.int32

    nc = bacc.Bacc("TRN2", target_bir_lowering=False, debug=False,
                   num_devices=N_CORES)

    NA = 2 * ((NPAIR + 1) // 2)       # 26 chain-A rows
    NB = S - NA                       # 24 chain-B rows
    NC = 32 + NB                      # compare layout height (56)

    NW2A = ((NPAIR + 1) // 2) * NA
    NW2B = (NPAIR // 2) * NB

    zin = nc.dram_tensor("z", [H + 2, W + 2, K, B_LOC], fp8,
                         kind="ExternalInput")
    w1v_d = nc.dram_tensor("w1v", [72, K * HID], fp8, kind="ExternalInput")
    w2p_d = nc.dram_tensor("w2pack", [128, NW2A + NW2B], bf16,
                           kind="ExternalInput")
    sp_d = nc.dram_tensor("smallpack", [128, 3], fp32, kind="ExternalInput")
    outp = nc.dram_tensor("out", [1, 1], fp32, kind="ExternalOutput")

    with tile.TileContext(nc) as tc:
        with (
            tc.tile_pool(name="const", bufs=1) as cpool,
            tc.tile_pool(name="ctx", bufs=24) as ctxpool,
            tc.tile_pool(name="hsb", bufs=8) as hpool,
            tc.tile_pool(name="ps", bufs=5, space="PSUM") as pspool,
            tc.tile_pool(name="psl", bufs=1, space="PSUM") as pslpool,
            tc.tile_pool(name="pst", bufs=1, space="PSUM") as pstpool,
        ):
            def emit_gather(s):
                v, i0, j0 = sites[s]
                ct = ctxpool.tile([72, B_LOC], fp8, tag="ctx")
                # ~15/17/18 split across the two HWDGE rings + SWDGE ring
                if s % 3 == 2 or s % 10 == 9:
                    eng = nc.gpsimd
                elif s % 3 == 0:
                    eng = nc.sync
                else:
                    eng = nc.scalar
                eng.dma_start(out=ct[:, :], in_=zin[i0:i0 + 3, j0:j0 + 3, :, :])
                return ct

            # critical path first: L1 weights + the first pairs' gathers
            w1v_sb = cpool.tile([72, K * HID], fp8)
            nc.sync.dma_start(out=w1v_sb[:, :], in_=w1v_d[:, :])
            ctx_q = {}
            for s in range(6):
                ctx_q[s] = emit_gather(s)

            # remaining constants (needed a few microseconds in)
            sp_sb = cpool.tile([128, 3], fp32)
            nc.scalar.dma_start(out=sp_sb[:, :], in_=sp_d[:, :])
            b1_sb = sp_sb[:, 0:1]
            ones_ap = sp_sb[0:NC, 1:2]
            tidx_ap = sp_sb[0:NC, 2:3].bitcast(i32)
            w2p_sb = cpool.tile([128, NW2A + NW2B], bf16)
            nc.scalar.dma_start(out=w2p_sb[:, :], in_=w2p_d[:, :])
            w2a_sb = w2p_sb[:, 0:NW2A]
            w2b_sb = w2p_sb[:, NW2A:NW2A + NW2B]

            # all target rows in one indirect gather (partitions 64+ to
            # touch the otherwise-idle high SBUF ports)
            tstage = cpool.tile([128, B_LOC], fp8)
            zflat = zin[:, :, :, :].rearrange("i j k b -> (i j k) b")
            nc.gpsimd.indirect_dma_start(
                out=tstage[64:64 + NC, :],
                out_offset=None,
                in_=zflat,
                in_offset=bass.IndirectOffsetOnAxis(ap=tidx_ap, axis=0),
            )
            t32 = cpool.tile([NC, B_LOC], fp32)
            nc.vector.tensor_copy(out=t32[:, :], in_=tstage[64:64 + NC, :])

            # two L2 accumulators in separate PSUM banks (interleaved
            # accumulation groups cannot share a bank: start=True clears the
            # whole bank's has_written bits)
            logA = pslpool.tile([128, B_LOC], fp32, tag="logA")
            logB = pslpool.tile([128, B_LOC], fp32, tag="logB")

            def emit_l2(pp, hs):
                if pp % 2 == 0:
                    e = pp // 2
                    outsl = logA[0:NA, :]
                    wsl = w2a_sb[:, e * NA:(e + 1) * NA]
                    col = 0
                else:
                    e = (pp - 1) // 2
                    outsl = logB[64:64 + NB, :]
                    wsl = w2b_sb[:, e * NB:(e + 1) * NB]
                    col = 64
                nc.tensor.matmul(
                    outsl, wsl, hs[:, :],
                    start=(pp in (0, 1)),
                    stop=(pp in (NPAIR - 1, NPAIR - 2)),
                    tile_position=(0, col),
                    skip_group_check=True)

            PF = 10  # pairs of gather prefetch
            for s in range(6, 2 * PF):
                ctx_q[s] = emit_gather(s)

            hq = {}
            for p in range(NPAIR):
                sA, sB = 2 * p, 2 * p + 1
                ctA = ctx_q.pop(sA)
                ctB = ctx_q.pop(sB)
                vA = sites[sA][0]
                vB = sites[sB][0]
                h_ps = pspool.tile([128, B_LOC], fp32, tag="hps")
                nc.tensor.matmul(
                    h_ps[0:HID, :], w1v_sb[:, vA * HID:(vA + 1) * HID],
                    ctA[:, :], start=True, stop=True, tile_position=(0, 0))
                nc.tensor.matmul(
                    h_ps[HID:128, :], w1v_sb[:, vB * HID:(vB + 1) * HID],
                    ctB[:, :], start=True, stop=True, tile_position=(0, 64))
                for s in (2 * (p + PF), 2 * (p + PF) + 1):
                    if s < S:
                        ctx_q[s] = emit_gather(s)
                h_sb = hpool.tile([128, B_LOC], bf16, tag="hsb")
                if p % 5 == 4:
                    nc.scalar.activation(
                        out=h_sb[:, :], in_=h_ps[:, :],
                        func=mybir.ActivationFunctionType.Relu,
                        bias=b1_sb[:, 0:1], scale=1.0)
                else:
                    nc.vector.tensor_scalar(
                        out=h_sb[:, :], in0=h_ps[:, :],
                        scalar1=b1_sb[:, 0:1], scalar2=0.0,
                        op0=mybir.AluOpType.add, op1=mybir.AluOpType.max)
                hq[p] = h_sb
                if p % 2 == 1:
                    emit_l2(p - 1, hq.pop(p - 1))
                    emit_l2(p, hq.pop(p))
            for pp in sorted(hq):
                emit_l2(pp, hq.pop(pp))

            # ((logit > -b2) != target) summed over batch, per chain
            junk = cpool.tile([NC, B_LOC], fp32)
            counts = cpool.tile([NC, 1], fp32)
            nc.vector.memset(counts[:, :], 0.0)
            nc.vector.scalar_tensor_tensor(
                out=junk[0:NA, :], in0=logA[0:NA, :], scalar=neg_b2,
                in1=t32[0:NA, :],
                op0=mybir.AluOpType.is_gt, op1=mybir.AluOpType.not_equal,
                accum_out=counts[0:NA, :])
            nc.vector.scalar_tensor_tensor(
                out=junk[32:NC, :], in0=logB[64:64 + NB, :], scalar=neg_b2,
                in1=t32[32:NC, :],
                op0=mybir.AluOpType.is_gt, op1=mybir.AluOpType.not_equal,
                accum_out=counts[32:NC, :])

            tot_ps = pstpool.tile([1, 1], fp32, tag="tot")
            nc.tensor.matmul(tot_ps[:, :], ones_ap, counts[:, :],
                             start=True, stop=True)
            res_sb = cpool.tile([1, 1], fp32)
            nc.vector.tensor_scalar_mul(res_sb[:, :], tot_ps[:, :],
                                        1.0 / float(B * S))
            nc.sync.dma_start(out=outp[:, :], in_=res_sb[:, :])

    nc.compile()
    return nc


def kernel(**inputs):
    global LAST_RESULTS
    from concourse.bass_utils import run_bass_kernel_spmd

    z = np.asarray(inputs["z"], dtype=np.float32)
    in_maps, sites, neg_b2 = _host_prep(
        z, np.asarray(inputs["W1"], dtype=np.float32),
        np.asarray(inputs["b1"], dtype=np.float32),
        np.asarray(inputs["W2"], dtype=np.float32),
        inputs["b2"], inputs["b_idx"], inputs["i_idx"], inputs["j_idx"])

    nc = _build_program(sites, neg_b2)

    res = run_bass_kernel_spmd(nc, in_maps, list(range(N_CORES)))
    LAST_RESULTS = res
    total = np.float32(0.0)
    for r in res.results:
        total += np.float32(r["out"][0, 0])
    return np.float32(total)
